# revision 3
# baseline (speedup 1.0000x reference)
"""Conv2d 3x3 (stride 1, pad 1) Trainium2 Bass kernel — Winograd F(2,3) along H.

Problem: x (32, 128, 56, 56) fp32, kernels (256, 128, 3, 3) fp32, b (256,) fp32
-> out (32, 256, 56, 56) fp32.

Strategy:
  - Data-parallel over batch: 32 images / 8 cores = 4 images per core. SPMD,
    no collectives.
  - Winograd F(2,3) applied along H (row pairs): the 3 kh taps collapse into
    4 components m_i = sum_kw U[i,kw]^T V_i(col-shift kw), so each pair of
    output rows costs 12 matmuls instead of 18 direct taps (2/3 the PE work;
    direct conv is PE-roofline-bound at ~92us/core, Winograd ~62us).
      V0 = x[2g-1]-x[2g+1], V1 = x[2g]+x[2g+1], V2 = -V1+2x[2g+1],
      V3 = x[2g]-x[2g+2]   (bf16, DVE 2x mode: inner step-1, row pairs are
      outer AP dims)
      y[2g]   = m0+m1+m2+b  (even output rows)
      y[2g+1] = m1-m2-m3+b  (odd output rows)
  - U = G-transform of the kernel along kh, precomputed on host in fp32,
    stored bf16 as 12 [128cin x 256cout] blocks ordered (i, kw=1,0,2).
  - Tiling: 7 row-pair groups per tile -> m_i tiles [128, 7, 56] fp32 = one
    PSUM bank each; 4 tiles x 2 cout-halves x 4 images = 32 accumulation
    groups of 12 matmuls (free dim 392/385).
  - Column zero-pad is implicit: kw=0/2 taps write ragged 55-col windows
    (kw=1 goes first with start=True covering the full window).
  - Eviction splits the output transform across engines so none exceeds the
    PE's ~1.96us per tile-half: ScalarE evicts m1+b, m2, m3 to bf16 SBUF;
    DVE computes p=s1+s2, u=s1-s2 (bf16 2x) and y_even=m0+p (PSUM read);
    GpSimd computes y_odd=u-s3. Bias rides the m1 eviction.
  - Startup: weights split over scalar/gpsimd DMA queues, x loaded as
    16-row halo chunks (first matmul gated only on chunk 0 + U part 0),
    warm-up matmuls lift the HAM clock gate during the load window.
"""

import numpy as np
import ml_dtypes

import concourse.bass as bass
import concourse.tile as tile
from concourse import bacc, mybir
from concourse.bass_utils import run_bass_kernel_spmd

N_CORES = 8
N_FULL = 32
N_PER = N_FULL // N_CORES  # 4 images per core
C_IN = 128
C_OUT = 256
H = W = 56
G = H // 2          # 28 row-pair groups
TK = 4              # m-tiles per image (along H)
GPT = G // TK       # 7 row-pair groups per tile
FD = GPT * W        # 392 <= 512 (one PSUM bank of fp32)

_DT = mybir.dt.bfloat16
_F32 = mybir.dt.float32
_ID = mybir.ActivationFunctionType.Identity

# Weight blocks ordered (comp i, kw in [1,0,2]); block bi holds [128cin, 256cout]
_KW_ORDER = [1, 0, 2]
_KW_POS = {1: 0, 0: 1, 2: 2}


def _build():
    nc = bacc.Bacc(
        "TRN2",
        target_bir_lowering=False,
        debug=False,
        num_devices=N_CORES,
    )
    xs = nc.dram_tensor("xs", [N_PER, C_IN, H, W], _DT, kind="ExternalInput").ap()
    wt = nc.dram_tensor("wt", [C_IN, 12 * C_OUT], _DT, kind="ExternalInput").ap()
    bt = nc.dram_tensor("bt", [128, 2], _F32, kind="ExternalInput").ap()
    y = nc.dram_tensor("y", [N_PER, C_OUT, H, W], _F32, kind="ExternalOutput").ap()

    with tile.TileContext(nc) as tc:
        with (
            tc.tile_pool(name="const", bufs=1) as const,
            tc.tile_pool(name="xpool", bufs=5) as xpool,
            tc.tile_pool(name="vpool", bufs=12) as vpool,
            tc.tile_pool(name="spool", bufs=10) as spool,
            tc.tile_pool(name="ypool", bufs=4) as ypool,
            tc.tile_pool(name="pspool", bufs=8, space="PSUM") as pspool,
        ):
            # PE warm-up: dummy matmuls depend on no DMA, so they run during
            # the input-load window and lift the HAM clock gate (1.2 -> 2.4
            # GHz) before real matmuls arrive.
            warm = const.tile([128, 512], _DT)
            nc.vector.memset(warm[:], 0.0)
            wps = pspool.tile([128, GPT, W], _F32, tag="ps", name="warm_ps")
            N_WARM = 9
            for i in range(N_WARM):
                nc.tensor.matmul(
                    wps[:],
                    lhsT=warm[:, :128],
                    rhs=warm[:, :FD].rearrange("p (g w) -> p g w", g=GPT),
                    start=(i == 0),
                    stop=(i == N_WARM - 1),
                )

            # Weights: two halves on two DMA queues so they land in parallel;
            # the first matmuls only wait on part 0 (comps 0-1).
            wt_sb0 = const.tile([C_IN, 6 * C_OUT], _DT)
            nc.scalar.dma_start(out=wt_sb0[:], in_=wt[:, : 6 * C_OUT])
            wt_sb1 = const.tile([C_IN, 6 * C_OUT], _DT)
            nc.gpsimd.dma_start(out=wt_sb1[:], in_=wt[:, 6 * C_OUT :])
            bias_sb = const.tile([128, 2], _F32)
            nc.scalar.dma_start(out=bias_sb[:], in_=bt)

            def wslice(i, kw, half):
                bi = i * 3 + _KW_POS[kw]
                sb = wt_sb0 if bi < 6 else wt_sb1
                c0 = (bi % 6) * C_OUT + half * 128
                return sb[:, c0 : c0 + 128]

            for n in range(N_PER):
                for k in range(TK):
                    # x chunk: global rows 14k-1 .. 14k+14 -> local rows 0..15
                    lo = max(0, 14 * k - 1)
                    hi = min(H, 14 * k + 15)
                    loff = lo - (14 * k - 1)
                    xc = xpool.tile([C_IN, 16, W], _DT, tag="xc", name=f"xc{n}_{k}")
                    nc.scalar.dma_start(
                        out=xc[:, loff : loff + (hi - lo), :], in_=xs[n, :, lo:hi, :]
                    )

                    # Input transform (DVE, bf16 2x): V_i [128, 7, 56]
                    v = [
                        vpool.tile([C_IN, GPT, W], _DT, tag="v", name=f"v{i}_{n}_{k}")
                        for i in range(4)
                    ]
                    # V1 = x[2g] + x[2g+1];  V2 = x[2g+1] - x[2g]
                    nc.vector.tensor_add(v[1][:], xc[:, 1:15:2, :], xc[:, 2:16:2, :])
                    nc.vector.tensor_sub(v[2][:], xc[:, 2:16:2, :], xc[:, 1:15:2, :])
                    # V0 = x[2g-1] - x[2g+1]  (g=0 of tile 0: row -1 is pad)
                    if k == 0:
                        nc.vector.tensor_sub(
                            v[0][:, 1:GPT, :], xc[:, 2:14:2, :], xc[:, 4:16:2, :]
                        )
                        nc.vector.tensor_scalar_mul(
                            v[0][:, 0:1, :], xc[:, 2:3, :], -1.0
                        )
                    else:
                        nc.vector.tensor_sub(
                            v[0][:], xc[:, 0:14:2, :], xc[:, 2:16:2, :]
                        )
                    # V3 = x[2g] - x[2g+2]  (g=27: row 56 is pad)
                    if k == TK - 1:
                        nc.vector.tensor_sub(
                            v[3][:, 0 : GPT - 1, :], xc[:, 1:13:2, :], xc[:, 3:15:2, :]
                        )
                        nc.vector.tensor_copy(v[3][:, GPT - 1 : GPT, :], xc[:, 13:14, :])
                    else:
                        nc.vector.tensor_sub(
                            v[3][:], xc[:, 1:15:2, :], xc[:, 3:16:2, :]
                        )

                    for half in range(2):
                        ms = {}
                        for i in (1, 2, 0, 3):
                            m = pspool.tile(
                                [128, GPT, W], _F32, tag="ps", name=f"m{i}_{n}_{k}_{half}"
                            )
                            ms[i] = m
                            for kw in _KW_ORDER:
                                if kw == 1:
                                    out_ap = m[:]
                                    rhs = v[i][:]
                                elif kw == 0:
                                    out_ap = m[:, :, 1:W]
                                    rhs = v[i][:, :, 0 : W - 1]
                                else:
                                    out_ap = m[:, :, 0 : W - 1]
                                    rhs = v[i][:, :, 1:W]
                                nc.tensor.matmul(
                                    out_ap,
                                    lhsT=wslice(i, kw, half),
                                    rhs=rhs,
                                    start=(kw == 1),
                                    stop=(kw == 2),
                                )

                        # Output transform:
                        #   y_even = m0 + (m1+b) + m2,  y_odd = (m1+b) - m2 - m3
                        s1 = spool.tile([128, GPT, W], _DT, tag="s", name=f"s1_{n}_{k}_{half}")
                        nc.scalar.activation(
                            s1[:], ms[1][:], _ID, bias=bias_sb[:, half : half + 1]
                        )
                        s2 = spool.tile([128, GPT, W], _DT, tag="s", name=f"s2_{n}_{k}_{half}")
                        nc.scalar.activation(s2[:], ms[2][:], _ID)
                        s3 = spool.tile([128, GPT, W], _DT, tag="s", name=f"s3_{n}_{k}_{half}")
                        nc.scalar.activation(s3[:], ms[3][:], _ID)

                        p = spool.tile([128, GPT, W], _DT, tag="s", name=f"p_{n}_{k}_{half}")
                        nc.vector.tensor_add(p[:], s1[:], s2[:])
                        u = spool.tile([128, GPT, W], _DT, tag="s", name=f"u_{n}_{k}_{half}")
                        nc.vector.tensor_sub(u[:], s1[:], s2[:])

                        yt = ypool.tile([128, 14, W], _F32, tag="yt", name=f"y_{n}_{k}_{half}")
                        nc.vector.tensor_add(yt[:, 0:14:2, :], ms[0][:], p[:])
                        nc.gpsimd.tensor_sub(yt[:, 1:14:2, :], u[:], s3[:])

                        y_slice = y[
                            n, half * 128 : (half + 1) * 128, 14 * k : 14 * k + 14, :
                        ]
                        if n == N_PER - 1 and half == 1 and k == TK - 1:
                            # split the final store so its DMA drain doesn't
                            # gate the end-of-kernel barrier on one queue
                            nc.sync.dma_start(
                                out=y_slice[:, 0:7, :], in_=yt[:, 0:7, :]
                            )
                            nc.scalar.dma_start(
                                out=y_slice[:, 7:14, :], in_=yt[:, 7:14, :]
                            )
                        else:
                            nc.sync.dma_start(out=y_slice, in_=yt[:])
    nc.compile()
    return nc


_NC = None


def _get_nc():
    global _NC
    if _NC is None:
        _NC = _build()
    return _NC


def _prep_inputs(x, kernels, b):
    bf16 = ml_dtypes.bfloat16
    xb = np.ascontiguousarray(x, dtype=np.float32).astype(bf16)
    w = np.asarray(kernels, dtype=np.float32)  # [O, C, kh, kw]
    U = [
        w[:, :, 0, :],
        0.5 * (w[:, :, 0, :] + w[:, :, 1, :] + w[:, :, 2, :]),
        0.5 * (w[:, :, 0, :] - w[:, :, 1, :] + w[:, :, 2, :]),
        w[:, :, 2, :],
    ]
    blocks = []
    for Ui in U:
        for kw in _KW_ORDER:
            blocks.append(Ui[:, :, kw].T)  # [C=128, O=256]
    wtb = np.ascontiguousarray(np.concatenate(blocks, axis=1)).astype(bf16)
    # bias [256] -> [128, 2]: column h holds b[h*128 : (h+1)*128]
    btb = np.ascontiguousarray(np.asarray(b, dtype=np.float32).reshape(2, 128).T)
    return xb, wtb, btb


def kernel(x, kernels, b):
    nc = _get_nc()
    xb, wtb, btb = _prep_inputs(x, kernels, b)
    in_maps = [
        {"xs": xb[i * N_PER : (i + 1) * N_PER], "wt": wtb, "bt": btb}
        for i in range(N_CORES)
    ]
    res = run_bass_kernel_spmd(nc, in_maps, core_ids=list(range(N_CORES)))
    out = np.concatenate(
        [r["y"].reshape(N_PER, C_OUT, H, W) for r in res.results], axis=0
    )
    return np.ascontiguousarray(out, dtype=np.float32)
